# revision 7
# baseline (speedup 1.0000x reference)
"""Multi-head attention (B=1, S=4096, D=1024, H=16, Hd=64) on 8 Trainium2 cores.

Sharding: tensor-parallel over heads - 2 heads per core. Each core computes
q/k/v projections for its 2 heads (128 dims), flash-style attention without
max-subtraction (scores are ~N(0,1) after scaling so exp never overflows),
and a partial output projection with its 128 rows of wo. Host sums the 8
partial outputs and adds bo.

All matmuls run as float32r (full-rate fp32 PE mode, ~1.5e-4 rel err).

Pipeline structure (v2): a single software-pipelined stream of
(Q-block, staged-group) work items. Q0's score/exp/ctx groups are emitted
*inside* the projection loop as soon as their k-chunks are projected, so the
ACT engine starts ~8us in instead of after all projections (~80us). Within
the stream, ctx matmuls trail score/exp emission by LAG groups (pending
deque), which removes PE head-of-line blocking and keeps ACT fed across
per-Q epilogues (normalize + out-proj on the freed ctx psum banks).

Layouts on device (per core):
  xT   [D, S]      streamed in blocks of [128 (d-chunk), 512 (s)]
  qT/kT[128, S]    partitions = head dims (h0: 0-63, h1: 64-127)
  v    [128, 2, 65] per k-chunk: partitions = seq rows, last col = ones
                   (so attn@v_aug also yields the softmax denominator)
  scores^T psum [128 (k rows), 3x512 (q)] -> exp on ACT (1536-wide)
  ctx^T psum [65, 512] per head, accumulated over 32 k-chunks
  out   [S, D]     natural layout, normalized via K=1 broadcast-matmul + recip
"""

import os
import sys
import types
from collections import deque

import numpy as np

S = 4096
D = 1024
H = 16
HD = 64
N_CORES = 8
HPC = H // N_CORES  # heads per core = 2
DC = D // 128       # d-chunks = 8
QB = 512            # q block

_LAST_EXEC_NS = None


def _install_ntff_hook_shim():
    if "antenv.axon_hooks" in sys.modules:
        return
    try:
        import antenv
        from trn_agent_boot.trn_boot import _ntff_profile_via_ctypes

        hook = _ntff_profile_via_ctypes("/opt/axon/libaxon_pjrt.so")
    except Exception:
        return
    mod = types.ModuleType("antenv.axon_hooks")
    _state = {"hook": hook}
    mod.get_axon_ntff_profile_hook = lambda: _state["hook"]
    mod.set_axon_ntff_profile_hook = lambda h: _state.update(hook=h)
    sys.modules["antenv.axon_hooks"] = mod
    antenv.axon_hooks = mod


def _build(s=S):
    import concourse.bass as bass
    import concourse.mybir as mybir
    import concourse.tile as tile
    from concourse import bacc
    from concourse.masks import make_identity

    f32 = mybir.dt.float32
    f32r = mybir.dt.float32r
    Exp = mybir.ActivationFunctionType.Exp

    KC = s // 128     # k-chunks = 32
    PB = 512          # projection block
    NP = s // PB      # projection blocks = 8
    GS = 3            # (kc, h) slices per exp staging group
    NSL = KC * HPC    # slices per Q block = 64
    NG = (NSL + GS - 1) // GS  # groups per Q block = 22
    LAG = 3           # ctx trails exp emission by this many groups
    EXBUFS = 6

    nc = bacc.Bacc("TRN2", target_bir_lowering=False, debug=False,
                   num_devices=N_CORES)

    NPb = s // 512
    xT_d = nc.declare_dram_parameter("xT", [NPb, 128, D // 128, 512], f32,
                                     isOutput=False)
    wq_d = nc.declare_dram_parameter("wq", [128, D], f32, isOutput=False)
    wk_d = nc.declare_dram_parameter("wk", [128, D], f32, isOutput=False)
    wv_d = nc.declare_dram_parameter("wv", [128, D], f32, isOutput=False)
    bq_d = nc.declare_dram_parameter("bq", [128, 1], f32, isOutput=False)
    bk_d = nc.declare_dram_parameter("bk", [128, 1], f32, isOutput=False)
    bv_d = nc.declare_dram_parameter("bv", [128, 1], f32, isOutput=False)
    wo_d = nc.declare_dram_parameter("wo", [128, D], f32, isOutput=False)
    out_d = nc.declare_dram_parameter("out", [s, D], f32, isOutput=True)

    with tile.TileContext(nc) as tc:
        import contextlib
        with contextlib.ExitStack() as ctx:
            wpool = ctx.enter_context(tc.tile_pool(name="w", bufs=1))
            xpool = ctx.enter_context(tc.tile_pool(name="x", bufs=2))
            kpool = ctx.enter_context(tc.tile_pool(name="kt", bufs=1))
            qpool = ctx.enter_context(tc.tile_pool(name="qt", bufs=NP))
            vpool = ctx.enter_context(tc.tile_pool(name="v4", bufs=KC))
            vtpool = ctx.enter_context(tc.tile_pool(name="vt", bufs=2))
            epool = ctx.enter_context(tc.tile_pool(name="ex", bufs=EXBUFS))
            cpool = ctx.enter_context(tc.tile_pool(name="ctxs", bufs=2))
            spool = ctx.enter_context(tc.tile_pool(name="sums", bufs=2))
            rpool = ctx.enter_context(tc.tile_pool(name="recb", bufs=2))
            opool = ctx.enter_context(tc.tile_pool(name="outs", bufs=3))
            # PSUM: stg = 2 x 3 banks (score staging, all projection psum,
            # v transposes); cp = 2 banks (ctx accumulators; reused for the
            # normalize broadcast + out-proj psum between accumulation epochs)
            stg = ctx.enter_context(tc.tile_pool(name="stg", bufs=2, space="PSUM"))
            cp = ctx.enter_context(tc.tile_pool(name="cp", bufs=1, space="PSUM"))

            # ---- constants / weights ----
            wq_t = wpool.tile([128, D], f32r, tag="wq")
            wk_t = wpool.tile([128, D], f32r, tag="wk")
            wv_t = wpool.tile([128, D], f32r, tag="wv")
            wo0_t = wpool.tile([64, D], f32r, tag="wo0")
            wo1_t = wpool.tile([64, D], f32r, tag="wo1")
            bq_t = wpool.tile([128, 1], f32, tag="bq")
            bk_t = wpool.tile([128, 1], f32, tag="bk")
            bv_t = wpool.tile([128, 1], f32, tag="bv")
            ident = wpool.tile([128, 128], f32, tag="ident")
            ones_f = wpool.tile([65, 64], f32, tag="ones_f")
            ones_t = wpool.tile([65, 64], f32r, tag="ones")
            onecol = wpool.tile([128, HPC, 1], f32, tag="onecol")

            nc.sync.dma_start(wq_t[:], wq_d[:].bitcast(f32r))
            nc.sync.dma_start(wk_t[:], wk_d[:].bitcast(f32r))
            nc.sync.dma_start(wv_t[:], wv_d[:].bitcast(f32r))
            nc.sync.dma_start(wo0_t[:], wo_d[0:64, :].bitcast(f32r))
            nc.sync.dma_start(wo1_t[:], wo_d[64:128, :].bitcast(f32r))
            nc.sync.dma_start(bq_t[:], bq_d[:])
            nc.sync.dma_start(bk_t[:], bk_d[:])
            nc.sync.dma_start(bv_t[:], bv_d[:])
            make_identity(nc, ident[:])
            nc.vector.memset(ones_f[:], 1.0)
            nc.vector.tensor_copy(ones_t[:], ones_f[:])
            nc.vector.memset(onecol[:], 1.0)

            kT = kpool.tile([128, s], f32r, tag="kT")
            q_tiles = [qpool.tile([128, PB], f32r, tag="qT",
                                  name="qT%d" % i)
                       for i in range(NP)]
            v_tiles = [None] * KC

            def mm(out, lhsT, rhs, start, stop):
                return nc.tensor.matmul(out, lhsT, rhs, start=start, stop=stop)

            def proj_block(w_t, dst_ap, bias_t, xb):
                ps = stg.tile([128, PB], f32, tag="stage")
                for c in range(DC):
                    mm(ps[:], w_t[:, c * 128:(c + 1) * 128], xb[:, c, :],
                       start=(c == 0), stop=(c == DC - 1))
                nc.vector.tensor_scalar_add(dst_ap, ps[:], bias_t[:])

            # flat (kc, h) slice list in GS-sized staging groups;
            # (kc,h0),(kc,h1) adjacent so K=64 row-tiled pairs overlap on PE
            slices = [(kc, h) for kc in range(KC) for h in range(HPC)]
            groups = [slices[i:i + GS] for i in range(0, len(slices), GS)]
            assert len(groups) == NG

            def emit_scores_exp(qb, gi):
                grp = groups[gi]
                ns = len(grp)
                st = stg.tile([128, GS, QB], f32, tag="stage")
                ex = epool.tile([128, GS, QB], f32r, tag="ex")
                for slot, (kc, h) in enumerate(grp):
                    mm(st[:, slot, :],
                       kT[h * 64:(h + 1) * 64, kc * 128:(kc + 1) * 128],
                       qb[h * 64:(h + 1) * 64, :],
                       start=True, stop=True)
                nc.scalar.activation(
                    ex[:, 0:ns, :], st[:, 0:ns, :], Exp,
                    bias=0.0, scale=float(1.0 / np.sqrt(HD)))
                return ex

            # per-Q ctx accumulators, created lazily at first ctx of each Q
            cur_ctx = [None, None]

            def ctx_group(gi, ex):
                if cur_ctx[0] is None:
                    cur_ctx[0] = cp.tile([65, QB], f32, tag="ctx0",
                                         name="ctxp0")
                    cur_ctx[1] = cp.tile([65, QB], f32, tag="ctx1",
                                         name="ctxp1")
                for slot, (kc, h) in enumerate(groups[gi]):
                    mm(cur_ctx[h][:], v_tiles[kc][:, h, :], ex[:, slot, :],
                       start=(kc == 0), stop=(kc == KC - 1))

            def epilogue(Q):
                ctxp0, ctxp1 = cur_ctx
                cur_ctx[0] = cur_ctx[1] = None
                # normalize: pull ctx + denominators out of psum, broadcast
                # the denominators via K=1 matmul, multiply by reciprocal
                cs0 = cpool.tile([64, QB], f32r, tag="cs0")
                cs1 = cpool.tile([64, QB], f32r, tag="cs1")
                sums = spool.tile([65, 2 * QB], f32r, tag="sums")
                nc.vector.tensor_copy(cs0[:], ctxp0[0:64, :])
                nc.vector.tensor_copy(cs1[:], ctxp1[0:64, :])
                nc.vector.tensor_copy(sums[64:65, 0:QB], ctxp0[64:65, :])
                nc.vector.tensor_copy(sums[64:65, QB:2 * QB], ctxp1[64:65, :])
                rb0 = cp.tile([64, QB], f32, tag="ctx0")
                rb1 = cp.tile([64, QB], f32, tag="ctx1")
                mm(rb0[:], ones_t[64:65, :], sums[64:65, 0:QB],
                   start=True, stop=True)
                mm(rb1[:], ones_t[64:65, :], sums[64:65, QB:2 * QB],
                   start=True, stop=True)
                rec = rpool.tile([64, 2, QB], f32, tag="rec")
                nc.vector.reciprocal_approx_fast(rec[:, 0, :], rb0[:])
                nc.vector.reciprocal_approx_fast(rec[:, 1, :], rb1[:])
                nc.vector.tensor_mul(cs0[:], cs0[:], rec[:, 0, :])
                nc.vector.tensor_mul(cs1[:], cs1[:], rec[:, 1, :])
                # out-proj: out[m-block, :] = cs0.T@wo0 + cs1.T@wo1
                for m in range(QB // 128):
                    for nh in range(D // 512):
                        op = cp.tile([128, 512], f32, tag="ctx%d" % (m % 2))
                        mm(op[:], cs0[:, m * 128:(m + 1) * 128],
                           wo0_t[:, nh * 512:(nh + 1) * 512],
                           start=True, stop=False)
                        mm(op[:], cs1[:, m * 128:(m + 1) * 128],
                           wo1_t[:, nh * 512:(nh + 1) * 512],
                           start=False, stop=True)
                        ob = opool.tile([128, 512], f32, tag="ob")
                        nc.vector.tensor_copy(ob[:], op[:])
                        nc.sync.dma_start(
                            out_d[Q * QB + m * 128:Q * QB + (m + 1) * 128,
                                  nh * 512:(nh + 1) * 512],
                            ob[:])

            # pending (Q, gi, ex) groups: exp emitted, ctx not yet emitted
            pend = deque()

            def pop_ctx():
                pq, pgi, pex = pend.popleft()
                ctx_group(pgi, pex)
                if pgi == NG - 1:
                    epilogue(pq)

            # ---- phase 1: projections with Q0 attention interleaved ----
            # after block b, k-chunks 0..4b+3 are projected; Q0 groups whose
            # slices all lie in that range can be emitted
            def q0_bound(b):
                if b >= NP - 1:
                    return NG
                return (8 * b + 5) // 3 + 1

            emitted = 0
            for b in range(NP):
                xb = xpool.tile([128, DC, PB], f32r, tag="xb")
                nc.sync.dma_start(xb[:], xT_d[b].bitcast(f32r))
                proj_block(wk_t, kT[:, b * PB:(b + 1) * PB], bk_t, xb)
                proj_block(wq_t, q_tiles[b][:], bq_t, xb)
                vt = vtpool.tile([128, PB], f32, tag="vt")
                proj_block(wv_t, vt[:], bv_t, xb)
                for j in range(PB // 128):
                    kc = b * (PB // 128) + j
                    tp = stg.tile([128, 128], f32, tag="stage")
                    nc.tensor.transpose(tp[:], vt[:, j * 128:(j + 1) * 128],
                                        ident[:])
                    v4 = vpool.tile([128, HPC, 65], f32r, tag="v4")
                    nc.vector.tensor_copy(v4[:, :, 64:65], onecol[:])
                    nc.vector.tensor_copy(
                        v4[:, :, 0:64],
                        tp[:].rearrange("p (h m) -> p h m", h=HPC))
                    v_tiles[kc] = v4
                # pump Q0: ctx trails emission so PE never head-of-line
                # blocks on the exp of the group just staged
                for gi in range(emitted, q0_bound(b)):
                    while len(pend) >= LAG:
                        pop_ctx()
                    pend.append((0, gi, emit_scores_exp(q_tiles[0], gi)))
                    emitted = gi + 1

            # ---- phase 2: Q1..Q7 attention, single software pipeline ----
            for Q in range(1, NP):
                for gi in range(NG):
                    while len(pend) >= LAG:
                        pop_ctx()
                    pend.append((Q, gi, emit_scores_exp(q_tiles[Q], gi)))
            while pend:
                pop_ctx()

    nc.compile()
    return nc


def _shard_inputs(x, wq, bq, wk, bk, wv, bv, wo, bo, s):
    # [D, s] -> contiguous per-block layout [s//512, 128, D//128, 512]
    xT2 = np.asarray(x, np.float32).reshape(s, D).T
    xT = np.ascontiguousarray(
        xT2.reshape(D // 128, 128, s // 512, 512).transpose(2, 1, 0, 3))

    def lhsT_layout(w, c):
        blk = np.asarray(w, np.float32)[:, c * 128:(c + 1) * 128]
        return np.ascontiguousarray(
            blk.reshape(DC, 128, 128).transpose(1, 0, 2).reshape(128, D))

    in_maps = []
    for c in range(N_CORES):
        in_maps.append({
            "xT": xT,
            "wq": lhsT_layout(wq, c),
            "wk": lhsT_layout(wk, c),
            "wv": lhsT_layout(wv, c),
            "bq": np.ascontiguousarray(
                np.asarray(bq, np.float32)[c * 128:(c + 1) * 128, None]),
            "bk": np.ascontiguousarray(
                np.asarray(bk, np.float32)[c * 128:(c + 1) * 128, None]),
            "bv": np.ascontiguousarray(
                np.asarray(bv, np.float32)[c * 128:(c + 1) * 128, None]),
            "wo": np.ascontiguousarray(
                np.asarray(wo, np.float32)[c * 128:(c + 1) * 128, :]),
        })
    return in_maps


def run(x, wq, bq, wk, bk, wv, bv, wo, bo, trace=False, s=S):
    global _LAST_EXEC_NS
    from concourse.bass_utils import run_bass_kernel_spmd

    if trace:
        _install_ntff_hook_shim()
    nc = _build(s)
    in_maps = _shard_inputs(x, wq, bq, wk, bk, wv, bv, wo, bo, s)
    res = run_bass_kernel_spmd(nc, in_maps, core_ids=list(range(N_CORES)),
                               trace=trace)
    _LAST_EXEC_NS = res.exec_time_ns
    out = res.results[0]["out"].astype(np.float64)
    for c in range(1, N_CORES):
        out += res.results[c]["out"]
    out += np.asarray(bo, np.float64)
    return out.astype(np.float32).reshape(1, s, D)


def kernel(x, wq, bq, wk, bk, wv, bv, wo, bo):
    trace = bool(os.environ.get("BASS_MHA_TRACE"))
    return run(x, wq, bq, wk, bk, wv, bv, wo, bo, trace=trace)


# revision 10
# speedup vs baseline: 1.1023x; 1.1023x over previous
"""Multi-head attention (B=1, S=4096, D=1024, H=16, Hd=64) on 8 Trainium2 cores.

Sharding: tensor-parallel over heads - 2 heads per core. Each core computes
q/k/v projections for its 2 heads (128 dims), flash-style attention without
max-subtraction (scores are ~N(0,1) after scaling so exp never overflows),
and a partial output projection with its 128 rows of wo. Host sums the 8
partial outputs and adds bo.

All matmuls run as float32r (full-rate fp32 PE mode, ~1.5e-4 rel err).

Pipeline structure (v2): a single software-pipelined stream of
(Q-block, staged-group) work items. Q0's score/exp/ctx groups are emitted
*inside* the projection loop as soon as their k-chunks are projected, so the
ACT engine starts ~8us in instead of after all projections (~80us). Within
the stream, ctx matmuls trail score/exp emission by LAG groups (pending
deque), which removes PE head-of-line blocking and keeps ACT fed across
per-Q epilogues (normalize + out-proj on the freed ctx psum banks).

Layouts on device (per core):
  xT   [D, S]      streamed in blocks of [128 (d-chunk), 512 (s)]
  qT/kT[128, S]    partitions = head dims (h0: 0-63, h1: 64-127)
  v    [128, 2, 65] per k-chunk: partitions = seq rows, last col = ones
                   (so attn@v_aug also yields the softmax denominator)
  scores^T psum [128 (k rows), 3x512 (q)] -> exp on ACT (1536-wide)
  ctx^T psum [65, 512] per head, accumulated over 32 k-chunks
  out   [S, D]     natural layout, normalized via K=1 broadcast-matmul + recip
"""

import os
import sys
import types
from collections import deque

import numpy as np

S = 4096
D = 1024
H = 16
HD = 64
N_CORES = 8
HPC = H // N_CORES  # heads per core = 2
DC = D // 128       # d-chunks = 8
QB = 512            # q block

_LAST_EXEC_NS = None


def _install_ntff_hook_shim():
    if "antenv.axon_hooks" in sys.modules:
        return
    try:
        import antenv
        from trn_agent_boot.trn_boot import _ntff_profile_via_ctypes

        hook = _ntff_profile_via_ctypes("/opt/axon/libaxon_pjrt.so")
    except Exception:
        return
    mod = types.ModuleType("antenv.axon_hooks")
    _state = {"hook": hook}
    mod.get_axon_ntff_profile_hook = lambda: _state["hook"]
    mod.set_axon_ntff_profile_hook = lambda h: _state.update(hook=h)
    sys.modules["antenv.axon_hooks"] = mod
    antenv.axon_hooks = mod


def _build(s=S):
    import concourse.bass as bass
    import concourse.mybir as mybir
    import concourse.tile as tile
    from concourse import bacc
    from concourse.masks import make_identity

    f32 = mybir.dt.float32
    f32r = mybir.dt.float32r
    Exp = mybir.ActivationFunctionType.Exp

    KC = s // 128     # k-chunks = 32
    PB = 512          # projection block
    NP = s // PB      # projection blocks = 8
    GS = 3            # (kc, h) slices per exp staging group
    NSL = KC * HPC    # slices per Q block = 64
    NG = (NSL + GS - 1) // GS  # groups per Q block = 22
    LAG = 3           # ctx trails exp emission by this many groups
    EXBUFS = 6

    nc = bacc.Bacc("TRN2", target_bir_lowering=False, debug=False,
                   num_devices=N_CORES)

    NPb = s // 512
    xT_d = nc.declare_dram_parameter("xT", [NPb, 128, D // 128, 512], f32,
                                     isOutput=False)
    wq_d = nc.declare_dram_parameter("wq", [128, D], f32, isOutput=False)
    wk_d = nc.declare_dram_parameter("wk", [128, D], f32, isOutput=False)
    wv_d = nc.declare_dram_parameter("wv", [128, D], f32, isOutput=False)
    bq_d = nc.declare_dram_parameter("bq", [128, 1], f32, isOutput=False)
    bk_d = nc.declare_dram_parameter("bk", [128, 1], f32, isOutput=False)
    bv_d = nc.declare_dram_parameter("bv", [128, 1], f32, isOutput=False)
    wo_d = nc.declare_dram_parameter("wo", [128, D], f32, isOutput=False)
    out_d = nc.declare_dram_parameter("out", [s, D], f32, isOutput=True)

    with tile.TileContext(nc) as tc:
        import contextlib
        with contextlib.ExitStack() as ctx:
            wpool = ctx.enter_context(tc.tile_pool(name="w", bufs=1))
            xpool = ctx.enter_context(tc.tile_pool(name="x", bufs=2))
            kpool = ctx.enter_context(tc.tile_pool(name="kt", bufs=1))
            qpool = ctx.enter_context(tc.tile_pool(name="qt", bufs=NP))
            vpool = ctx.enter_context(tc.tile_pool(name="v4", bufs=KC))
            vtpool = ctx.enter_context(tc.tile_pool(name="vt", bufs=2))
            epool = ctx.enter_context(tc.tile_pool(name="ex", bufs=EXBUFS))
            cpool = ctx.enter_context(tc.tile_pool(name="ctxs", bufs=2))
            spool = ctx.enter_context(tc.tile_pool(name="sums", bufs=2))
            rpool = ctx.enter_context(tc.tile_pool(name="recb", bufs=2))
            opool = ctx.enter_context(tc.tile_pool(name="outs", bufs=3))
            # PSUM: stg = 2 x 3 banks (score staging, all projection psum,
            # v transposes); cp = 2 banks (ctx accumulators; reused for the
            # normalize broadcast + out-proj psum between accumulation epochs)
            stg = ctx.enter_context(tc.tile_pool(name="stg", bufs=2, space="PSUM"))
            cp = ctx.enter_context(tc.tile_pool(name="cp", bufs=1, space="PSUM"))

            # ---- constants / weights ----
            wq_t = wpool.tile([128, D], f32r, tag="wq")
            wk_t = wpool.tile([128, D], f32r, tag="wk")
            wv_t = wpool.tile([128, D], f32r, tag="wv")
            wo0_t = wpool.tile([64, D], f32r, tag="wo0")
            wo1_t = wpool.tile([64, D], f32r, tag="wo1")
            bq_t = wpool.tile([128, 1], f32, tag="bq")
            bk_t = wpool.tile([128, 1], f32, tag="bk")
            bv_t = wpool.tile([128, 1], f32, tag="bv")
            ident = wpool.tile([128, 128], f32, tag="ident")
            ones_f = wpool.tile([65, 64], f32, tag="ones_f")
            ones_t = wpool.tile([65, 64], f32r, tag="ones")
            onecol = wpool.tile([128, HPC, 1], f32, tag="onecol")

            # k-proj inputs first so the first projection can start while the
            # remaining weights stream in behind them
            nc.sync.dma_start(wk_t[:], wk_d[:].bitcast(f32r))
            nc.sync.dma_start(bk_t[:], bk_d[:])
            make_identity(nc, ident[:])
            nc.vector.memset(ones_f[:], 1.0)
            nc.vector.tensor_copy(ones_t[:], ones_f[:])
            nc.vector.memset(onecol[:], 1.0)

            def load_late_weights():
                nc.sync.dma_start(wq_t[:], wq_d[:].bitcast(f32r))
                nc.sync.dma_start(bq_t[:], bq_d[:])
                nc.sync.dma_start(wv_t[:], wv_d[:].bitcast(f32r))
                nc.sync.dma_start(bv_t[:], bv_d[:])
                nc.sync.dma_start(wo0_t[:], wo_d[0:64, :].bitcast(f32r))
                nc.sync.dma_start(wo1_t[:], wo_d[64:128, :].bitcast(f32r))

            kT = kpool.tile([128, s], f32r, tag="kT")
            q_tiles = [qpool.tile([128, PB], f32r, tag="qT",
                                  name="qT%d" % i)
                       for i in range(NP)]
            v_tiles = [None] * KC

            def mm(out, lhsT, rhs, start, stop):
                return nc.tensor.matmul(out, lhsT, rhs, start=start, stop=stop)

            def proj_block(w_t, dst_ap, bias_t, xb):
                ps = stg.tile([128, PB], f32, tag="stage")
                for c in range(DC):
                    mm(ps[:], w_t[:, c * 128:(c + 1) * 128], xb[:, c, :],
                       start=(c == 0), stop=(c == DC - 1))
                nc.vector.tensor_scalar_add(dst_ap, ps[:], bias_t[:])

            # flat (kc, h) slice list in GS-sized staging groups;
            # (kc,h0),(kc,h1) adjacent so K=64 row-tiled pairs overlap on PE
            slices = [(kc, h) for kc in range(KC) for h in range(HPC)]
            groups = [slices[i:i + GS] for i in range(0, len(slices), GS)]
            assert len(groups) == NG

            def emit_scores_exp(qb, gi):
                grp = groups[gi]
                ns = len(grp)
                st = stg.tile([128, GS, QB], f32, tag="stage")
                ex = epool.tile([128, GS, QB], f32r, tag="ex")
                for slot, (kc, h) in enumerate(grp):
                    mm(st[:, slot, :],
                       kT[h * 64:(h + 1) * 64, kc * 128:(kc + 1) * 128],
                       qb[h * 64:(h + 1) * 64, :],
                       start=True, stop=True)
                nc.scalar.activation(
                    ex[:, 0:ns, :], st[:, 0:ns, :], Exp,
                    bias=0.0, scale=float(1.0 / np.sqrt(HD)))
                return ex

            # per-Q ctx accumulators, created lazily at first ctx of each Q
            cur_ctx = [None, None]

            def ctx_group(gi, ex):
                if cur_ctx[0] is None:
                    cur_ctx[0] = cp.tile([65, QB], f32, tag="ctx0",
                                         name="ctxp0")
                    cur_ctx[1] = cp.tile([65, QB], f32, tag="ctx1",
                                         name="ctxp1")
                for slot, (kc, h) in enumerate(groups[gi]):
                    mm(cur_ctx[h][:], v_tiles[kc][:, h, :], ex[:, slot, :],
                       start=(kc == 0), stop=(kc == KC - 1))

            # Epilogue is split: the psum->sbuf copies run at the Q boundary
            # (freeing the ctx psum banks for the next Q), while the
            # broadcast/normalize/out-proj matmuls are deferred and dribbled
            # out between later staging groups so they never form a long
            # PE-FIFO block that starves ACT. Their psum comes from the stg
            # pool (cp is busy accumulating the next Q by then).
            deferred = deque()

            def epilogue_head(Q):
                ctxp0, ctxp1 = cur_ctx
                cur_ctx[0] = cur_ctx[1] = None
                cs0 = cpool.tile([64, QB], f32r, tag="cs0")
                cs1 = cpool.tile([64, QB], f32r, tag="cs1")
                sums = spool.tile([65, 2 * QB], f32r, tag="sums")
                nc.vector.tensor_copy(cs0[:], ctxp0[0:64, :])
                nc.vector.tensor_copy(cs1[:], ctxp1[0:64, :])
                nc.vector.tensor_copy(sums[64:65, 0:QB], ctxp0[64:65, :])
                nc.vector.tensor_copy(sums[64:65, QB:2 * QB], ctxp1[64:65, :])

                def norm_chunk():
                    rbb = stg.tile([64, 2, QB], f32, tag="stage")
                    mm(rbb[:, 0, :], ones_t[64:65, :], sums[64:65, 0:QB],
                       start=True, stop=True)
                    mm(rbb[:, 1, :], ones_t[64:65, :], sums[64:65, QB:2 * QB],
                       start=True, stop=True)
                    rec = rpool.tile([64, 2, QB], f32, tag="rec")
                    nc.vector.reciprocal_approx_fast(rec[:, 0, :], rbb[:, 0, :])
                    nc.vector.reciprocal_approx_fast(rec[:, 1, :], rbb[:, 1, :])
                    nc.vector.tensor_mul(cs0[:], cs0[:], rec[:, 0, :])
                    nc.vector.tensor_mul(cs1[:], cs1[:], rec[:, 1, :])

                deferred.append(norm_chunk)

                def op_chunk(m, nh):
                    def emit():
                        op = stg.tile([128, QB], f32, tag="stage")
                        mm(op[:], cs0[:, m * 128:(m + 1) * 128],
                           wo0_t[:, nh * 512:(nh + 1) * 512],
                           start=True, stop=False)
                        mm(op[:], cs1[:, m * 128:(m + 1) * 128],
                           wo1_t[:, nh * 512:(nh + 1) * 512],
                           start=False, stop=True)
                        ob = opool.tile([128, 512], f32, tag="ob")
                        nc.vector.tensor_copy(ob[:], op[:])
                        nc.sync.dma_start(
                            out_d[Q * QB + m * 128:Q * QB + (m + 1) * 128,
                                  nh * 512:(nh + 1) * 512],
                            ob[:])
                    return emit

                for m in range(QB // 128):
                    for nh in range(D // 512):
                        deferred.append(op_chunk(m, nh))

            # pending (Q, gi, ex) groups: exp emitted, ctx not yet emitted
            pend = deque()

            def pop_ctx():
                pq, pgi, pex = pend.popleft()
                ctx_group(pgi, pex)
                if pgi == NG - 1:
                    epilogue_head(pq)

            # ---- phase 1: projections with Q0 attention interleaved ----
            # after block b, k-chunks 0..4b+3 are projected; Q0 groups whose
            # slices all lie in that range can be emitted
            def q0_bound(b):
                if b >= NP - 1:
                    return NG
                return (8 * b + 5) // 3 + 1

            emitted = 0
            for b in range(NP):
                xb = xpool.tile([128, DC, PB], f32r, tag="xb")
                # chunked so the first d-chunk's k-proj starts immediately
                for c in range(DC):
                    nc.sync.dma_start(xb[:, c, :], xT_d[b, :, c, :].bitcast(f32r))
                proj_block(wk_t, kT[:, b * PB:(b + 1) * PB], bk_t, xb)
                if b == 0:
                    load_late_weights()
                proj_block(wq_t, q_tiles[b][:], bq_t, xb)
                vt = vtpool.tile([128, PB], f32, tag="vt")
                proj_block(wv_t, vt[:], bv_t, xb)
                for j in range(PB // 128):
                    kc = b * (PB // 128) + j
                    tp = stg.tile([128, 128], f32, tag="stage")
                    nc.tensor.transpose(tp[:], vt[:, j * 128:(j + 1) * 128],
                                        ident[:])
                    v4 = vpool.tile([128, HPC, 65], f32r, tag="v4")
                    nc.vector.tensor_copy(v4[:, :, 64:65], onecol[:])
                    nc.vector.tensor_copy(
                        v4[:, :, 0:64],
                        tp[:].rearrange("p (h m) -> p h m", h=HPC))
                    v_tiles[kc] = v4
                # pump Q0: ctx trails emission so PE never head-of-line
                # blocks on the exp of the group just staged
                for gi in range(emitted, q0_bound(b)):
                    while len(pend) >= LAG:
                        pop_ctx()
                    pend.append((0, gi, emit_scores_exp(q_tiles[0], gi)))
                    emitted = gi + 1

            # ---- phase 2: Q1..Q7 attention, single software pipeline ----
            # every other group, dribble out one deferred epilogue chunk
            step = 0
            for Q in range(1, NP):
                for gi in range(NG):
                    while len(pend) >= LAG:
                        pop_ctx()
                    pend.append((Q, gi, emit_scores_exp(q_tiles[Q], gi)))
                    step += 1
                    if deferred and step % 2 == 0:
                        deferred.popleft()()
            while pend:
                pop_ctx()
            while deferred:
                deferred.popleft()()

    nc.compile()
    return nc


def _shard_inputs(x, wq, bq, wk, bk, wv, bv, wo, bo, s):
    # [D, s] -> contiguous per-block layout [s//512, 128, D//128, 512]
    xT2 = np.asarray(x, np.float32).reshape(s, D).T
    xT = np.ascontiguousarray(
        xT2.reshape(D // 128, 128, s // 512, 512).transpose(2, 1, 0, 3))

    def lhsT_layout(w, c):
        blk = np.asarray(w, np.float32)[:, c * 128:(c + 1) * 128]
        return np.ascontiguousarray(
            blk.reshape(DC, 128, 128).transpose(1, 0, 2).reshape(128, D))

    in_maps = []
    for c in range(N_CORES):
        in_maps.append({
            "xT": xT,
            "wq": lhsT_layout(wq, c),
            "wk": lhsT_layout(wk, c),
            "wv": lhsT_layout(wv, c),
            "bq": np.ascontiguousarray(
                np.asarray(bq, np.float32)[c * 128:(c + 1) * 128, None]),
            "bk": np.ascontiguousarray(
                np.asarray(bk, np.float32)[c * 128:(c + 1) * 128, None]),
            "bv": np.ascontiguousarray(
                np.asarray(bv, np.float32)[c * 128:(c + 1) * 128, None]),
            "wo": np.ascontiguousarray(
                np.asarray(wo, np.float32)[c * 128:(c + 1) * 128, :]),
        })
    return in_maps


def run(x, wq, bq, wk, bk, wv, bv, wo, bo, trace=False, s=S):
    global _LAST_EXEC_NS
    from concourse.bass_utils import run_bass_kernel_spmd

    if trace:
        _install_ntff_hook_shim()
    nc = _build(s)
    in_maps = _shard_inputs(x, wq, bq, wk, bk, wv, bv, wo, bo, s)
    res = run_bass_kernel_spmd(nc, in_maps, core_ids=list(range(N_CORES)),
                               trace=trace)
    _LAST_EXEC_NS = res.exec_time_ns
    out = res.results[0]["out"].astype(np.float64)
    for c in range(1, N_CORES):
        out += res.results[c]["out"]
    out += np.asarray(bo, np.float64)
    return out.astype(np.float32).reshape(1, s, D)


def kernel(x, wq, bq, wk, bk, wv, bv, wo, bo):
    trace = bool(os.environ.get("BASS_MHA_TRACE"))
    return run(x, wq, bq, wk, bk, wv, bv, wo, bo, trace=trace)
